# revision 1
# baseline (speedup 1.0000x reference)
"""BlockSparseAttention TRN2 kernel — 8-core SPMD (batch x head-half sharding).

Reference computation (B=4, S=2048, D=1024, H=16, Dh=64):
  q/k/v = x @ W{q,k,v}.T + b   -> [B,H,S,Dh]
  scores = q k^T / 8, masked to |i-j|<=32 plus global rows/cols (<4)
  out = softmax(scores) v  -> reassembled -> @ Wo.T + bo

Sharding: core c handles batch b=c//2, head-group g=c%2 (heads 8g..8g+7,
channels 512g..512g+511). Each core computes its heads' attention output and
a PARTIAL o-projection (contraction over its 512 channels); host sums the two
partials per batch and adds bo.

On-chip layout is fully transposed (d-major): projections compute q_T/k_T
directly as [chan, s]; scores are computed transposed [t, s] so softmax sums
land in a matmul ones-row (v augmented with a ones column) and no transposes
are ever needed. Attention works on s-tiles of 256 with 128-aligned t-chunks;
within each chunk only the s-slice intersecting the band is computed.
"""
import numpy as np
import ml_dtypes

import concourse.bass as bass
import concourse.bacc as bacc
import concourse.tile as tile
import concourse.mybir as mybir
from concourse.bass_utils import run_bass_kernel_spmd

F32 = mybir.dt.float32
BF16 = mybir.dt.bfloat16
AF = mybir.ActivationFunctionType
ALU = mybir.AluOpType

S = 2048
D = 1024
NCORES = 8
NEG = -1.0e9
SCALE = 0.125


def chunk_plan():
    """Per s-tile r (256 rows): list of (t0, lo, w): 128-wide t-chunk at t0,
    contributing to local s columns [lo, lo+w)."""
    plans = []
    for r in range(8):
        if r == 0:
            t0s = [0, 128, 256]
        elif r == 7:
            t0s = [1664, 1792, 1920]
        else:
            t0s = [256 * r - 128, 256 * r, 256 * r + 128, 256 * r + 256]
        cur = []
        for j, t0 in enumerate(t0s):
            if r == 0 and j == 0:
                lo, hi = 0, 256  # global cols t<4 make every s valid
            else:
                lo = max(0, t0 - 32 - 256 * r)
                hi = min(256, t0 + 160 - 256 * r)
            cur.append((t0, lo, hi - lo))
        plans.append(cur)
    return plans


PLANS = chunk_plan()
MASK_OFF = []  # flat offsets into packed masks tensor, in (r, j) order
_off = 0
for _r in range(8):
    _row = []
    for (_t0, _lo, _w) in PLANS[_r]:
        _row.append(_off)
        _off += _w
    MASK_OFF.append(_row)
MASK_W = _off  # total packed width


def build_nc():
    nc = bacc.Bacc()
    xT = nc.dram_tensor("xT", [128, 8, S], BF16, kind="ExternalInput")
    wq = nc.dram_tensor("wq", [128, 8, 512], BF16, kind="ExternalInput")
    wk = nc.dram_tensor("wk", [128, 8, 512], BF16, kind="ExternalInput")
    wv = nc.dram_tensor("wv", [128, 8, 512], BF16, kind="ExternalInput")
    wo = nc.dram_tensor("wo", [128, 4, 1024], BF16, kind="ExternalInput")
    bq_c = nc.dram_tensor("bq_c", [128, 4], F32, kind="ExternalInput")
    bk_c = nc.dram_tensor("bk_c", [128, 4], F32, kind="ExternalInput")
    bv_b = nc.dram_tensor("bv_b", [128, 512], F32, kind="ExternalInput")
    masks = nc.dram_tensor("masks", [128, MASK_W], BF16, kind="ExternalInput")
    out = nc.dram_tensor("out", [128, 8, S], F32, kind="ExternalOutput")

    with tile.TileContext(nc) as tc:
        with (
            tc.tile_pool(name="pers", bufs=1) as pers,
            tc.tile_pool(name="small", bufs=1) as small,
        ):
            q_sb = pers.tile([128, 4, S], BF16)
            k_sb = pers.tile([128, 4, S], BF16)
            v_sb = pers.tile([128, 16, 520], BF16)
            att_sb = pers.tile([128, 4, S], BF16)
            masks_sb = pers.tile([128, MASK_W], BF16)
            nc.sync.dma_start(out=masks_sb, in_=masks.ap())
            bq_sb = small.tile([128, 4], F32)
            bk_sb = small.tile([128, 4], F32)
            bv_sb = small.tile([128, 512], F32)
            nc.sync.dma_start(out=bq_sb, in_=bq_c.ap())
            nc.sync.dma_start(out=bk_sb, in_=bk_c.ap())
            nc.sync.dma_start(out=bv_sb, in_=bv_b.ap())

            # ---------------- Phase 1: projections ----------------
            with (
                tc.tile_pool(name="wpool", bufs=1) as wpool,
                tc.tile_pool(name="xpool", bufs=4) as xpool,
                tc.tile_pool(name="pproj", bufs=8, space="PSUM") as pproj,
            ):
                wq_sb = wpool.tile([128, 8, 512], BF16, tag="wq")
                wk_sb = wpool.tile([128, 8, 512], BF16, tag="wk")
                wv_sb = wpool.tile([128, 8, 512], BF16, tag="wv")
                nc.sync.dma_start(out=wq_sb, in_=wq.ap())
                nc.sync.dma_start(out=wk_sb, in_=wk.ap())
                nc.sync.dma_start(out=wv_sb, in_=wv.ap())

                for st in range(4):
                    ssl = slice(st * 512, (st + 1) * 512)
                    pq = [pproj.tile([128, 512], F32, tag="proj", name=f"pq{st}_{i}")
                          for i in range(4)]
                    pk = [pproj.tile([128, 512], F32, tag="proj", name=f"pk{st}_{i}")
                          for i in range(4)]
                    for dc in range(8):
                        xt = xpool.tile([128, 512], BF16, tag="xa")
                        nc.sync.dma_start(out=xt, in_=xT.ap()[:, dc, ssl])
                        for cb in range(4):
                            csl = slice(cb * 128, (cb + 1) * 128)
                            nc.tensor.matmul(
                                pq[cb], wq_sb[:, dc, csl], xt,
                                start=(dc == 0), stop=(dc == 7))
                            nc.tensor.matmul(
                                pk[cb], wk_sb[:, dc, csl], xt,
                                start=(dc == 0), stop=(dc == 7))
                    for cb in range(4):
                        nc.vector.tensor_scalar(
                            out=q_sb[:, cb, ssl], in0=pq[cb],
                            scalar1=bq_sb[:, cb:cb + 1], scalar2=None, op0=ALU.add)
                        nc.vector.tensor_scalar(
                            out=k_sb[:, cb, ssl], in0=pk[cb],
                            scalar1=bk_sb[:, cb:cb + 1], scalar2=None, op0=ALU.add)
                    pv = [pproj.tile([128, 512], F32, tag="proj", name=f"pv{st}_{i}")
                          for i in range(4)]
                    for dc in range(8):
                        xt = xpool.tile([128, 512], BF16, tag="xb")
                        nc.sync.dma_start(out=xt, in_=xT.ap()[:, dc, ssl])
                        for s4 in range(4):
                            nc.tensor.matmul(
                                pv[s4], xt[:, s4 * 128:(s4 + 1) * 128],
                                wv_sb[:, dc, :],
                                start=(dc == 0), stop=(dc == 7))
                    for s4 in range(4):
                        sc = st * 4 + s4
                        vview = v_sb[:, sc, :].rearrange("p (h w) -> p h w", h=8)
                        nc.vector.tensor_add(
                            out=vview[:, :, 0:64],
                            in0=pv[s4].rearrange("p (h w) -> p h w", h=8),
                            in1=bv_sb.rearrange("p (h w) -> p h w", h=8))
                        nc.vector.memset(vview[:, :, 64:65], 1.0)

                # HAM warm burst: ~5us of chained dense matmuls as the LAST
                # PE work of the projection phase, bridging the PSUM pool
                # handover stall so the clock gate stays at 8/8.
                wt = pproj.tile([128, 512], F32, tag="proj", name="warm")
                for i in range(24):
                    nc.tensor.matmul(
                        wt, k_sb[:, 0, 0:128], q_sb[:, 0, 0:512],
                        start=(i == 0), stop=(i == 23))
                wt_sb = small.tile([128, 4], F32, name="wt_sb")
                nc.vector.tensor_copy(out=wt_sb, in_=wt[:, 0:4])
                nc.vector.tensor_scalar(
                    out=att_sb[:, 0, 0:4], in0=wt_sb, scalar1=0.0,
                    scalar2=None, op0=ALU.mult)

            # ------- Phase 2: attention, with o-proj interleaved as PE filler -------
            with (
                tc.tile_pool(name="psc", bufs=3, space="PSUM") as psc,
                tc.tile_pool(name="paug", bufs=3, space="PSUM") as paug,
                tc.tile_pool(name="pso", bufs=2, space="PSUM") as pso,
                tc.tile_pool(name="epool", bufs=6) as epool,
                tc.tile_pool(name="rpool", bufs=4) as rpool,
                tc.tile_pool(name="wop", bufs=1) as wop,
                tc.tile_pool(name="opool", bufs=3) as opool,
            ):
                wo_sb = wop.tile([128, 4, 1024], BF16)
                nc.sync.dma_start(out=wo_sb, in_=wo.ap())
                import os as _os
                WARM = _os.environ.get('KWARM', '1') == '1'

                # HAM warmup: a chained burst of dense matmuls (~5us of
                # continuous PE work) to lift the clock gate to 8/8 right as
                # the attention phase begins; kept alive by a dummy DMA.

                def band_pair(r, hp, augp):
                    """Both heads of the pair into one aug bank; per-head
                    packed scores -> one mask-add -> one exp -> AVs."""
                    rsl = slice(r * 256, (r + 1) * 256)
                    offs = []
                    acc = 0
                    for (_t0, _lo, _w) in PLANS[r]:
                        offs.append(acc)
                        acc += _w
                    wr = acc
                    moff = MASK_OFF[r][0]
                    for hs in (0, 64):
                        half = (hs // 64) * 256
                        h65 = (hp * 2 + hs // 64) * 65
                        first = hs == 0
                        nav = 0
                        exs = None
                        if r > 0:
                            strip = psc.tile([4, 256], F32, tag="sc",
                                             name=f"st{r}{hp}{hs}")
                            nc.tensor.matmul(
                                strip, k_sb[hs:hs + 64, hp, 0:4],
                                q_sb[hs:hs + 64, hp, rsl],
                                start=True, stop=True)
                            exs = epool.tile([4, 256], BF16, tag="exps",
                                             name=f"xs{r}{hp}{hs}")
                            nc.scalar.activation(exs, strip, AF.Exp,
                                                 scale=SCALE)
                        sct = psc.tile([128, 512], F32, tag="sc",
                                       name=f"sc{r}{hp}{hs}")
                        for j, (t0, lo, w) in enumerate(PLANS[r]):
                            ssl2 = slice(r * 256 + lo, r * 256 + lo + w)
                            nc.tensor.matmul(
                                sct[:, offs[j]:offs[j] + w],
                                k_sb[hs:hs + 64, hp, t0:t0 + 128],
                                q_sb[hs:hs + 64, hp, ssl2],
                                start=(j == 0), stop=(j == len(PLANS[r]) - 1),
                                skip_group_check=True)
                        scf = epool.tile([128, 512], F32, tag="scf",
                                         name=f"sf{r}{hp}{hs}")
                        nc.vector.tensor_add(
                            out=scf[:, 0:wr], in0=sct[:, 0:wr],
                            in1=masks_sb[:, moff:moff + wr])
                        ex = epool.tile([128, 512], BF16, tag="exp",
                                        name=f"ex{r}{hp}{hs}")
                        nc.scalar.activation(
                            ex[:, 0:wr], scf[:, 0:wr], AF.Exp, scale=SCALE)
                        if r > 0:
                            nc.tensor.matmul(
                                augp[:, half:half + 256],
                                v_sb[0:4, 0, h65:h65 + 65], exs,
                                start=first, stop=False, skip_group_check=True)
                            nav += 1
                        for j, (t0, lo, w) in enumerate(PLANS[r]):
                            nc.tensor.matmul(
                                augp[:, half + lo:half + lo + w],
                                v_sb[:, t0 // 128, h65:h65 + 65],
                                ex[:, offs[j]:offs[j] + w],
                                start=(first and nav == 0),
                                stop=(r > 0 and hs == 64
                                      and j == len(PLANS[r]) - 1),
                                skip_group_check=True)
                            nav += 1

                def norm_pair(r, hp, augp):
                    """att = augp[0:64] / augp[64] for both heads of the pair."""
                    rsl = slice(r * 256, (r + 1) * 256)
                    for hs in (0, 64):
                        half = (hs // 64) * 256
                        nc.scalar.copy(out=att_sb[hs:hs + 64, hp, rsl],
                                       in_=augp[0:64, half:half + 256])
                    sums = rpool.tile([1, 512], F32, tag="sums", name=f"su{r}{hp}")
                    nc.vector.tensor_copy(out=sums, in_=augp[64:65, :])
                    rec = rpool.tile([1, 512], F32, tag="rec", name=f"re{r}{hp}")
                    nc.vector.reciprocal_approx_fast(out=rec, in_=sums)
                    bc = rpool.tile([128, 512], F32, tag="bc", name=f"bc{r}{hp}")
                    nc.gpsimd.partition_broadcast(bc, rec)
                    for hs in (0, 64):
                        half = (hs // 64) * 256
                        nc.vector.tensor_mul(
                            out=att_sb[hs:hs + 64, hp, rsl],
                            in0=att_sb[hs:hs + 64, hp, rsl],
                            in1=bc[hs:hs + 64, half:half + 256])

                def oproj_block(st, ets):
                    ssl = slice(st * 512, (st + 1) * 512)
                    for et in ets:
                        esl = slice(et * 128, (et + 1) * 128)
                        po = pso.tile([128, 512], F32, tag="po",
                                      name=f"po{st}_{et}")
                        for cb in range(4):
                            nc.tensor.matmul(
                                po, wo_sb[:, cb, esl], att_sb[:, cb, ssl],
                                start=(cb == 0), stop=(cb == 3))
                        otq = opool.tile([128, 512], F32, tag="otq",
                                         name=f"otq{st}_{et}")
                        nc.vector.tensor_copy(out=otq, in_=po)
                        nc.sync.dma_start(out=out.ap()[:, et, ssl], in_=otq)

                for r in range(8):
                    for hp in range(4):
                        augp = paug.tile([65, 512], F32, tag="aug",
                                         name=f"au{r}{hp}")
                        band_pair(r, hp, augp)
                        if r == 0:
                            for hs in (0, 64):
                                h65 = (hp * 2 + hs // 64) * 65
                                half = (hs // 64) * 256
                                gsc = psc.tile([128, 64], F32, tag="sc",
                                               name=f"gs{hp}{hs}")
                                for kk in range(16):
                                    nc.tensor.matmul(
                                        gsc[:, 4 * kk:4 * kk + 4],
                                        k_sb[hs:hs + 64, hp,
                                             128 * kk:128 * kk + 128],
                                        q_sb[hs:hs + 64, hp, 0:4],
                                        start=(kk == 0), stop=(kk == 15))
                                exg = epool.tile([128, 64], BF16, tag="expg",
                                                 name=f"xg{hp}{hs}")
                                nc.scalar.activation(exg, gsc, AF.Exp, scale=SCALE)
                                for kk in range(16):
                                    nc.tensor.matmul(
                                        augp[:, half:half + 4],
                                        v_sb[:, kk, h65:h65 + 65],
                                        exg[:, 4 * kk:4 * kk + 4],
                                        start=False,
                                        stop=(hs == 64 and kk == 15),
                                        skip_group_check=True)
                        norm_pair(r, hp, augp)
                    if r % 2 == 1:
                        # 8-et o-proj block (~7us of dense N=512 matmuls)
                        # after every other s-tile: lifts the HAM clock gate.
                        oproj_block(r // 2, range(8))

    nc.compile()
    return nc


def _host_masks():
    p = np.arange(128)[:, None]
    tiles = np.empty((128, MASK_W), np.float32)
    for r in range(8):
        for j, (t0, lo, w) in enumerate(PLANS[r]):
            sl = np.arange(lo, lo + w)[None, :]
            s = 256 * r + sl
            t = t0 + p
            valid = (s >= 4) & ((np.abs(t - s) <= 32) | (t < 4))
            mo = MASK_OFF[r][j]
            tiles[:, mo:mo + w] = np.where(valid, 0.0, NEG)
    return tiles.astype(ml_dtypes.bfloat16)


_NC = None
_LAST_IN_MAPS = None


def kernel(x, Wq, bq, Wk, bk, Wv, bv, Wo, bo):
    global _NC
    if _NC is None:
        _NC = build_nc()
    nc = _NC
    x = np.asarray(x, np.float32)
    B = x.shape[0]
    bf = ml_dtypes.bfloat16

    def chunked_T(a):  # [R, C] -> [128, C//128, R]; [p, c, r] = a[r, 128c+p]
        at = np.ascontiguousarray(a.T)
        return at.reshape(at.shape[0] // 128, 128, at.shape[1]).transpose(1, 0, 2)

    masks_h = _host_masks()
    in_maps = []
    for core in range(NCORES):
        b, g = core // 2, core % 2
        gs = slice(512 * g, 512 * (g + 1))
        in_maps.append({
            "xT": np.ascontiguousarray(chunked_T(x[b])).astype(bf),
            "wq": np.ascontiguousarray(chunked_T(np.asarray(Wq)[gs, :])).astype(bf),
            "wk": np.ascontiguousarray(chunked_T(np.asarray(Wk)[gs, :])).astype(bf),
            "wv": np.ascontiguousarray(chunked_T(np.asarray(Wv)[gs, :])).astype(bf),
            "wo": np.ascontiguousarray(chunked_T(np.asarray(Wo)[:, gs])).astype(bf),
            "bq_c": np.asarray(bq)[gs].reshape(4, 128).T.copy().astype(np.float32),
            "bk_c": np.asarray(bk)[gs].reshape(4, 128).T.copy().astype(np.float32),
            "bv_b": np.broadcast_to(
                np.asarray(bv)[gs], (128, 512)).copy().astype(np.float32),
            "masks": masks_h,
        })

    global _LAST_IN_MAPS
    _LAST_IN_MAPS = in_maps
    res = run_bass_kernel_spmd(nc, in_maps, list(range(NCORES)))
    out = np.empty((B, S, D), np.float32)
    for b in range(B):
        acc = res.results[2 * b]["out"].astype(np.float32) + \
            res.results[2 * b + 1]["out"].astype(np.float32)
        full_T = acc.transpose(1, 0, 2).reshape(D, S)
        out[b] = full_T.T + np.asarray(bo)[None, :]
    return out



# revision 9
# speedup vs baseline: 1.2022x; 1.2022x over previous
"""BlockSparseAttention TRN2 kernel — 8-core SPMD (batch x head-half sharding).

Reference computation (B=4, S=2048, D=1024, H=16, Dh=64):
  q/k/v = x @ W{q,k,v}.T + b   -> [B,H,S,Dh]
  scores = q k^T / 8, masked to |i-j|<=32 plus global rows/cols (<4)
  out = softmax(scores) v  -> reassembled -> @ Wo.T + bo

Sharding: core c handles batch b=c//2, head-group g=c%2 (heads 8g..8g+7,
channels 512g..512g+511). Each core computes its heads' attention output and
a PARTIAL o-projection (contraction over its 512 channels); host sums the two
partials per batch and adds bo.

On-chip layout is fully transposed (d-major): projections compute q_T/k_T
directly as [chan, s]; scores are computed transposed [t, s] so softmax sums
land in a matmul ones-row (v augmented with a ones column).

v2 design notes (vs v1):
  - x is SBUF-resident, DMA'd once; projections run cb-outer so q/k/v fit in
    a 4-bank PSUM rotation and form one continuous dense PE stream.
  - Attention is a 1-deep software pipeline over the 32 (r, hp) pairs:
    scores(j) | bc(j-2) | AV(j-1) per iteration, with PE work grouped by
    tiling mode (64-row scores w/ h0/h64 row-tile concurrency, 32-row
    broadcast MMs, 128-row AVs) to minimize PE array mode-switch drains.
  - Global (t<4) key columns are computed as a 5th band-style chunk with a
    0/1 bf16 mask MULTIPLY after exp (no -1e9 add pass, no 4-partition strip
    matmuls, no 32-row AV section).
  - Softmax normalization: denominator row -> bf16 copy -> PE K=1 broadcast
    matmul (col-tiled into the att layout) -> reciprocal_approx_fast on the
    [128, 256] broadcast -> one [128, 256] multiply. No GpSimd broadcast.
  - o-projection emitted as dense N=512 blocks after the norm of each s-tile
    pair; output stored bf16 (host sums partials in f32).
"""
import numpy as np
import ml_dtypes

import concourse.bass as bass
import concourse.bacc as bacc
import concourse.tile as tile
import concourse.mybir as mybir
from concourse.bass_utils import run_bass_kernel_spmd

F32 = mybir.dt.float32
BF16 = mybir.dt.bfloat16
AF = mybir.ActivationFunctionType
ALU = mybir.AluOpType

S = 2048
D = 1024
NCORES = 8
SCALE = 0.125

# ---------------------------------------------------------------------------
# Chunk plans: per s-tile r (256 rows), the score/AV chunks.
# Each chunk: (sc, exoff, lo, w, mkind) where sc = 128-row v/t chunk index,
# exoff = column offset in the packed ex tile, lo = s-local start, w = width,
# mkind identifies the mask pattern.
#   mkind: 'G' (global cols t<4), 'D' (delta=-128), 'A' (delta=0),
#          'B' (delta=+128), 'C' (delta=+256), 'S' (r=0 special j0)
# sctA holds the first 448 ex columns, sctB the rest (<=192).
# ---------------------------------------------------------------------------


def chunk_plan(r):
    if r == 0:
        return [(0, 0, 0, 256, 'S'), (1, 256, 96, 160, 'B'),
                (2, 416, 224, 32, 'C')]
    if r == 7:
        return [(0, 0, 0, 256, 'G'), (13, 256, 0, 32, 'D'),
                (14, 288, 0, 160, 'A'), (15, 448, 96, 160, 'B')]
    return [(0, 0, 0, 256, 'G'), (2 * r - 1, 256, 0, 32, 'D'),
            (2 * r, 288, 0, 160, 'A'), (2 * r + 1, 448, 96, 160, 'B'),
            (2 * r + 2, 608, 224, 32, 'C')]


CHUNKS = [chunk_plan(r) for r in range(8)]
WR = [sum(c[3] for c in CHUNKS[r]) for r in range(8)]  # 448 / 640 / 608
A_COLS = 448
# mask packing: one column-block per r-class, matching ex layout
MOFF = {0: 640, 7: 1088}  # interior at 0
MASK_W = 640 + 448 + 608


def mask_off(r):
    return MOFF.get(r, 0)


def build_nc():
    nc = bacc.Bacc()
    xT = nc.dram_tensor("xT", [128, 8, S], BF16, kind="ExternalInput")
    wq = nc.dram_tensor("wq", [128, 8, 512], BF16, kind="ExternalInput")
    wk = nc.dram_tensor("wk", [128, 8, 512], BF16, kind="ExternalInput")
    wv = nc.dram_tensor("wv", [128, 8, 512], BF16, kind="ExternalInput")
    wo = nc.dram_tensor("wo", [128, 4, 1024], BF16, kind="ExternalInput")
    bq_c = nc.dram_tensor("bq_c", [128, 4], F32, kind="ExternalInput")
    bk_c = nc.dram_tensor("bk_c", [128, 4], F32, kind="ExternalInput")
    bv_b = nc.dram_tensor("bv_b", [128, 512], F32, kind="ExternalInput")
    masks = nc.dram_tensor("masks", [128, MASK_W], BF16, kind="ExternalInput")
    out = nc.dram_tensor("out", [128, 8, S], BF16, kind="ExternalOutput")

    with tile.TileContext(nc) as tc:
        with (
            tc.tile_pool(name="pers", bufs=1) as pers,
            tc.tile_pool(name="small", bufs=1) as small,
        ):
            x_sb = pers.tile([128, 8, S], BF16)
            q_sb = pers.tile([128, 4, S], BF16)
            k_sb = pers.tile([128, 4, S], BF16)
            v_sb = pers.tile([128, 16, 520], BF16)
            att_sb = pers.tile([128, 4, S], BF16)
            masks_sb = pers.tile([128, MASK_W], BF16)
            wq_sb = pers.tile([128, 8, 512], BF16)
            wk_sb = pers.tile([128, 8, 512], BF16)
            wv_sb = pers.tile([128, 8, 512], BF16)
            wo_sb = pers.tile([128, 4, 1024], BF16)
            bq_sb = small.tile([128, 4], F32)
            bk_sb = small.tile([128, 4], F32)
            bv_sb = small.tile([128, 512], F32)
            ones_sb = small.tile([1, 128], BF16)
            nc.vector.memset(ones_sb, 1.0)

            # DMA issue order tuned so the first q-projection chain can start
            # ~4us in: wq + biases first, then x for st=0, then the rest.
            nc.sync.dma_start(out=wq_sb, in_=wq.ap())
            nc.sync.dma_start(out=bq_sb, in_=bq_c.ap())
            nc.sync.dma_start(out=bk_sb, in_=bk_c.ap())
            for dc in range(8):
                nc.sync.dma_start(out=x_sb[:, dc, 0:512], in_=xT.ap()[:, dc, 0:512])
            nc.sync.dma_start(out=wk_sb, in_=wk.ap())
            nc.sync.dma_start(out=wv_sb, in_=wv.ap())
            nc.sync.dma_start(out=bv_sb, in_=bv_b.ap())
            for dc in range(8):
                nc.sync.dma_start(out=x_sb[:, dc, 512:1024],
                                  in_=xT.ap()[:, dc, 512:1024])
            nc.sync.dma_start(out=masks_sb, in_=masks.ap())
            for st in (2, 3):
                ssl = slice(st * 512, (st + 1) * 512)
                for dc in range(8):
                    nc.sync.dma_start(out=x_sb[:, dc, ssl], in_=xT.ap()[:, dc, ssl])
            nc.sync.dma_start(out=wo_sb, in_=wo.ap())

            # ---------------- Phase 1: projections (dense, x-resident) -----
            with tc.tile_pool(name="pproj", bufs=4, space="PSUM") as pproj:
                for st in range(4):
                    ssl = slice(st * 512, (st + 1) * 512)
                    for cb in range(4):
                        csl = slice(cb * 128, (cb + 1) * 128)
                        pq = pproj.tile([128, 512], F32, tag="proj",
                                        name=f"pq{st}_{cb}")
                        for dc in range(8):
                            nc.tensor.matmul(
                                pq, wq_sb[:, dc, csl], x_sb[:, dc, ssl],
                                start=(dc == 0), stop=(dc == 7))
                        nc.vector.tensor_scalar(
                            out=q_sb[:, cb, ssl], in0=pq,
                            scalar1=bq_sb[:, cb:cb + 1], scalar2=None,
                            op0=ALU.add)
                    for cb in range(4):
                        csl = slice(cb * 128, (cb + 1) * 128)
                        pk = pproj.tile([128, 512], F32, tag="proj",
                                        name=f"pk{st}_{cb}")
                        for dc in range(8):
                            nc.tensor.matmul(
                                pk, wk_sb[:, dc, csl], x_sb[:, dc, ssl],
                                start=(dc == 0), stop=(dc == 7))
                        nc.vector.tensor_scalar(
                            out=k_sb[:, cb, ssl], in0=pk,
                            scalar1=bk_sb[:, cb:cb + 1], scalar2=None,
                            op0=ALU.add)
                    for s4 in range(4):
                        xsl = slice(st * 512 + s4 * 128, st * 512 + s4 * 128 + 128)
                        pv = pproj.tile([128, 512], F32, tag="proj",
                                        name=f"pv{st}_{s4}")
                        for dc in range(8):
                            nc.tensor.matmul(
                                pv, x_sb[:, dc, xsl], wv_sb[:, dc, :],
                                start=(dc == 0), stop=(dc == 7))
                        sc = st * 4 + s4
                        vview = v_sb[:, sc, :].rearrange("p (h w) -> p h w", h=8)
                        nc.vector.tensor_add(
                            out=vview[:, :, 0:64],
                            in0=pv.rearrange("p (h w) -> p h w", h=8),
                            in1=bv_sb.rearrange("p (h w) -> p h w", h=8))
                        nc.vector.memset(vview[:, :, 64:65], 1.0)

            # ---------------- Phase 2: attention pipeline ------------------
            with (
                tc.tile_pool(name="psctA", bufs=2, space="PSUM") as psctA,
                tc.tile_pool(name="psctB", bufs=2, space="PSUM") as psctB,
                tc.tile_pool(name="paug", bufs=2, space="PSUM") as paug,
                # bc (normalizer broadcast) and po (o-proj) share one
                # 2-slot rotation: 8 PSUM banks total, and their lifetimes
                # interleave without stalls.
                tc.tile_pool(name="pmix", bufs=2, space="PSUM") as pmix,
                tc.tile_pool(name="epool", bufs=4) as epool,
                tc.tile_pool(name="rpool", bufs=2) as rpool,
                tc.tile_pool(name="opool", bufs=3) as opool,
            ):
                PAIRS = [(r, hp) for r in range(8) for hp in range(4)]
                state = {}  # j -> dict of tiles

                def emit_scores(j):
                    r, hp = PAIRS[j]
                    st = {}
                    b_cols = WR[r] - A_COLS
                    for hs in (0, 64):
                        st[f"sctA{hs}"] = psctA.tile(
                            [128, 512], F32, tag="sctA", name=f"sA{j}_{hs}")
                        if b_cols > 0:
                            st[f"sctB{hs}"] = psctB.tile(
                                [128, 192], F32, tag="sctB", name=f"sB{j}_{hs}")
                    # interleave h0/h64 chunk by chunk: row-tile concurrency
                    ca = [c for c in CHUNKS[r] if c[1] < A_COLS]
                    cbl = [c for c in CHUNKS[r] if c[1] >= A_COLS]
                    for ci, (sc, exoff, lo, w, mk) in enumerate(ca):
                        for hs in (0, 64):
                            nc.tensor.matmul(
                                st[f"sctA{hs}"][:, exoff:exoff + w],
                                k_sb[hs:hs + 64, hp, sc * 128:sc * 128 + 128],
                                q_sb[hs:hs + 64, hp,
                                     r * 256 + lo:r * 256 + lo + w],
                                start=(ci == 0), stop=(ci == len(ca) - 1),
                                skip_group_check=True)
                    for ci, (sc, exoff, lo, w, mk) in enumerate(cbl):
                        off = exoff - A_COLS
                        for hs in (0, 64):
                            nc.tensor.matmul(
                                st[f"sctB{hs}"][:, off:off + w],
                                k_sb[hs:hs + 64, hp, sc * 128:sc * 128 + 128],
                                q_sb[hs:hs + 64, hp,
                                     r * 256 + lo:r * 256 + lo + w],
                                start=(ci == 0), stop=(ci == len(cbl) - 1),
                                skip_group_check=True)
                    if r == 0:
                        for hs in (0, 64):
                            st[f"gsc{hs}"] = psctB.tile(
                                [128, 64], F32, tag="sctB", name=f"gs{j}_{hs}")
                        for kk in range(16):
                            for hs in (0, 64):
                                nc.tensor.matmul(
                                    st[f"gsc{hs}"][:, 4 * kk:4 * kk + 4],
                                    k_sb[hs:hs + 64, hp,
                                         128 * kk:128 * kk + 128],
                                    q_sb[hs:hs + 64, hp, 0:4],
                                    start=(kk == 0), stop=(kk == 15),
                                    skip_group_check=True)
                    state[j] = st

                def emit_exps(j):
                    r, hp = PAIRS[j]
                    st = state[j]
                    wr = WR[r]
                    b_cols = wr - A_COLS
                    for hs in (0, 64):
                        ex = epool.tile([128, 640], BF16, tag="ex",
                                        name=f"ex{j}_{hs}")
                        st[f"ex{hs}"] = ex
                        nc.scalar.activation(
                            ex[:, 0:A_COLS], st[f"sctA{hs}"][:, 0:A_COLS],
                            AF.Exp, scale=SCALE)
                        if b_cols > 0:
                            nc.scalar.activation(
                                ex[:, A_COLS:wr], st[f"sctB{hs}"][:, 0:b_cols],
                                AF.Exp, scale=SCALE)
                        if r == 0:
                            exg = epool.tile([128, 64], BF16, tag="exg",
                                             name=f"xg{j}_{hs}")
                            st[f"exg{hs}"] = exg
                            nc.scalar.activation(exg, st[f"gsc{hs}"],
                                                 AF.Exp, scale=SCALE)

                def emit_maskmuls(j):
                    r, hp = PAIRS[j]
                    st = state[j]
                    wr = WR[r]
                    mo = mask_off(r)
                    for hs in (0, 64):
                        nc.vector.tensor_mul(
                            out=st[f"ex{hs}"][:, 0:wr],
                            in0=st[f"ex{hs}"][:, 0:wr],
                            in1=masks_sb[:, mo:mo + wr])

                def emit_avs(j):
                    r, hp = PAIRS[j]
                    st = state[j]
                    aug = paug.tile([65, 512], F32, tag="aug", name=f"au{j}")
                    st["aug"] = aug
                    n_ch = len(CHUNKS[r])
                    for hi, hs in enumerate((0, 64)):
                        half = (hs // 64) * 256
                        h65 = (hp * 2 + hs // 64) * 65
                        ex = st[f"ex{hs}"]
                        for ci, (sc, exoff, lo, w, mk) in enumerate(CHUNKS[r]):
                            last = (r != 0 and hi == 1 and ci == n_ch - 1)
                            nc.tensor.matmul(
                                aug[:, half + lo:half + lo + w],
                                v_sb[:, sc, h65:h65 + 65],
                                ex[:, exoff:exoff + w],
                                start=(hi == 0 and ci == 0), stop=last,
                                skip_group_check=True)
                    if r == 0:
                        for hi, hs in enumerate((0, 64)):
                            half = (hs // 64) * 256
                            h65 = (hp * 2 + hs // 64) * 65
                            exg = st[f"exg{hs}"]
                            for kk in range(16):
                                nc.tensor.matmul(
                                    aug[:, half:half + 4],
                                    v_sb[:, kk, h65:h65 + 65],
                                    exg[:, 4 * kk:4 * kk + 4],
                                    start=False,
                                    stop=(hi == 1 and kk == 15),
                                    skip_group_check=True)

                def emit_den(j):
                    st = state[j]
                    den = epool.tile([1, 512], BF16, tag="den", name=f"dn{j}")
                    st["den"] = den
                    nc.vector.tensor_copy(out=den, in_=st["aug"][64:65, :])

                def emit_attcopies(j):
                    r, hp = PAIRS[j]
                    st = state[j]
                    rsl = slice(r * 256, (r + 1) * 256)
                    for hs in (0, 64):
                        half = (hs // 64) * 256
                        nc.scalar.copy(out=att_sb[hs:hs + 64, hp, rsl],
                                       in_=st["aug"][0:64, half:half + 256])

                def emit_bc(j):
                    st = state[j]
                    bc = pmix.tile([128, 256], F32, tag="pobc", name=f"bc{j}")
                    st["bc"] = bc
                    den = st["den"]
                    nc.tensor.matmul(bc[0:64, :], ones_sb[0:1, 0:64],
                                     den[0:1, 0:256], start=True, stop=True)
                    nc.tensor.matmul(bc[64:128, :], ones_sb[0:1, 0:64],
                                     den[0:1, 256:512], start=True, stop=True,
                                     tile_position=(0, 64))
                    del st["den"]

                def emit_recmul(j):
                    r, hp = PAIRS[j]
                    st = state[j]
                    rsl = slice(r * 256, (r + 1) * 256)
                    rec = rpool.tile([128, 256], F32, tag="rec", name=f"rc{j}")
                    nc.vector.reciprocal_approx_fast(out=rec, in_=st["bc"])
                    nc.vector.tensor_mul(
                        out=att_sb[:, hp, rsl], in0=att_sb[:, hp, rsl],
                        in1=rec)
                    state.pop(j, None)

                def oproj_block(stq):
                    ssl = slice(stq * 512, (stq + 1) * 512)
                    for et in range(8):
                        esl = slice(et * 128, (et + 1) * 128)
                        po = pmix.tile([128, 512], F32, tag="pobc",
                                      name=f"po{stq}_{et}")
                        for cb in range(4):
                            nc.tensor.matmul(
                                po, wo_sb[:, cb, esl], att_sb[:, cb, ssl],
                                start=(cb == 0), stop=(cb == 3))
                        otq = opool.tile([128, 512], BF16, tag="otq",
                                         name=f"otq{stq}_{et}")
                        nc.vector.tensor_copy(out=otq, in_=po)
                        nc.sync.dma_start(out=out.ap()[:, et, ssl], in_=otq)

                for j in range(32):
                    emit_scores(j)
                    if j >= 2:
                        emit_bc(j - 2)
                    if j >= 1:
                        emit_avs(j - 1)
                    if j >= 2:
                        emit_recmul(j - 2)
                    emit_exps(j)
                    emit_maskmuls(j)
                    if j >= 1:
                        emit_den(j - 1)
                        emit_attcopies(j - 1)
                    if j >= 9 and (j - 9) % 8 == 0 and (j - 9) // 8 < 3:
                        oproj_block((j - 9) // 8)
                # flush
                emit_avs(31)
                emit_den(31)
                emit_attcopies(31)
                emit_bc(30)
                emit_recmul(30)
                emit_bc(31)
                emit_recmul(31)
                oproj_block(3)

    nc.compile()
    return nc


def _host_masks():
    p = np.arange(128)[:, None]

    def band(delta, lo, w):
        sl = np.arange(w)[None, :]
        return (np.abs(delta + p - lo - sl) <= 32).astype(np.float32)

    def gcols(w):
        sl = np.arange(w)[None, :]
        return ((p < 4) + 0 * sl).astype(np.float32)

    def special(w):  # r=0 j0: t in [0,128), s=sl
        sl = np.arange(w)[None, :]
        return ((sl >= 4) & ((np.abs(p - sl) <= 32) | (p < 4))).astype(np.float32)

    interior = np.concatenate(
        [gcols(256), band(-128, 0, 32), band(0, 0, 160),
         band(128, 96, 160), band(256, 224, 32)], axis=1)
    r0 = np.concatenate(
        [special(256), band(128, 96, 160), band(256, 224, 32)], axis=1)
    r7 = np.concatenate(
        [gcols(256), band(-128, 0, 32), band(0, 0, 160),
         band(128, 96, 160)], axis=1)
    full = np.concatenate([interior, r0, r7], axis=1)
    assert full.shape == (128, MASK_W)
    return full.astype(ml_dtypes.bfloat16)


_NC = None
_LAST_IN_MAPS = None


def kernel(x, Wq, bq, Wk, bk, Wv, bv, Wo, bo):
    global _NC
    if _NC is None:
        _NC = build_nc()
    nc = _NC
    x = np.asarray(x, np.float32)
    B = x.shape[0]
    bf = ml_dtypes.bfloat16

    def chunked_T(a):  # [R, C] -> [128, C//128, R]; [p, c, r] = a[r, 128c+p]
        at = np.ascontiguousarray(a.T)
        return at.reshape(at.shape[0] // 128, 128, at.shape[1]).transpose(1, 0, 2)

    masks_h = _host_masks()
    in_maps = []
    for core in range(NCORES):
        b, g = core // 2, core % 2
        gs = slice(512 * g, 512 * (g + 1))
        in_maps.append({
            "xT": np.ascontiguousarray(chunked_T(x[b])).astype(bf),
            "wq": np.ascontiguousarray(chunked_T(np.asarray(Wq)[gs, :])).astype(bf),
            "wk": np.ascontiguousarray(chunked_T(np.asarray(Wk)[gs, :])).astype(bf),
            "wv": np.ascontiguousarray(chunked_T(np.asarray(Wv)[gs, :])).astype(bf),
            "wo": np.ascontiguousarray(chunked_T(np.asarray(Wo)[:, gs])).astype(bf),
            "bq_c": np.asarray(bq)[gs].reshape(4, 128).T.copy().astype(np.float32),
            "bk_c": np.asarray(bk)[gs].reshape(4, 128).T.copy().astype(np.float32),
            "bv_b": np.broadcast_to(
                np.asarray(bv)[gs], (128, 512)).copy().astype(np.float32),
            "masks": masks_h,
        })

    global _LAST_IN_MAPS
    _LAST_IN_MAPS = in_maps
    res = run_bass_kernel_spmd(nc, in_maps, list(range(NCORES)))
    out = np.empty((B, S, D), np.float32)
    for b in range(B):
        acc = res.results[2 * b]["out"].astype(np.float32) + \
            res.results[2 * b + 1]["out"].astype(np.float32)
        full_T = acc.transpose(1, 0, 2).reshape(D, S)
        out[b] = full_T.T + np.asarray(bo)[None, :]
    return out


# revision 11
# speedup vs baseline: 1.2805x; 1.0651x over previous
"""BlockSparseAttention TRN2 kernel — 8-core SPMD (batch x head-half sharding).

Reference computation (B=4, S=2048, D=1024, H=16, Dh=64):
  q/k/v = x @ W{q,k,v}.T + b   -> [B,H,S,Dh]
  scores = q k^T / 8, masked to |i-j|<=32 plus global rows/cols (<4)
  out = softmax(scores) v  -> reassembled -> @ Wo.T + bo

Sharding: core c handles batch b=c//2, head-group g=c%2 (heads 8g..8g+7,
channels 512g..512g+511). Each core computes its heads' attention output and
a PARTIAL o-projection (contraction over its 512 channels); host sums the two
partials per batch and adds bo.

On-chip layout is fully transposed (d-major): projections compute q_T/k_T
directly as [chan, s]; scores are computed transposed [t, s] so softmax sums
land in a matmul ones-row (v augmented with a ones column).

v2 design notes (vs v1):
  - x is SBUF-resident, DMA'd once; projections run cb-outer so q/k/v fit in
    a 4-bank PSUM rotation and form one continuous dense PE stream.
  - Attention is a 1-deep software pipeline over the 32 (r, hp) pairs:
    scores(j) | bc(j-2) | AV(j-1) per iteration, with PE work grouped by
    tiling mode (64-row scores w/ h0/h64 row-tile concurrency, 32-row
    broadcast MMs, 128-row AVs) to minimize PE array mode-switch drains.
  - Global (t<4) key columns are computed as a 5th band-style chunk with a
    0/1 bf16 mask MULTIPLY after exp (no -1e9 add pass, no 4-partition strip
    matmuls, no 32-row AV section).
  - Softmax normalization: denominator row -> bf16 copy -> PE K=1 broadcast
    matmul (col-tiled into the att layout) -> reciprocal_approx_fast on the
    [128, 256] broadcast -> one [128, 256] multiply. No GpSimd broadcast.
  - o-projection emitted as dense N=512 blocks after the norm of each s-tile
    pair; output stored bf16 (host sums partials in f32).
"""
import numpy as np
import ml_dtypes

import concourse.bass as bass
import concourse.bacc as bacc
import concourse.tile as tile
import concourse.mybir as mybir
from concourse.bass_utils import run_bass_kernel_spmd

F32 = mybir.dt.float32
BF16 = mybir.dt.bfloat16
AF = mybir.ActivationFunctionType
ALU = mybir.AluOpType

S = 2048
D = 1024
NCORES = 8
SCALE = 0.125

# ---------------------------------------------------------------------------
# Chunk plans: per s-tile r (256 rows), the score/AV chunks.
# Each chunk: (sc, exoff, lo, w, mkind) where sc = 128-row v/t chunk index,
# exoff = column offset in the packed ex tile, lo = s-local start, w = width,
# mkind identifies the mask pattern.
#   mkind: 'G' (global cols t<4), 'D' (delta=-128), 'A' (delta=0),
#          'B' (delta=+128), 'C' (delta=+256), 'S' (r=0 special j0)
# sctA holds the first 448 ex columns, sctB the rest (<=192).
# ---------------------------------------------------------------------------


def chunk_plan(r):
    if r == 0:
        return [(0, 0, 0, 256, 'S'), (1, 256, 96, 160, 'B'),
                (2, 416, 224, 32, 'C')]
    if r == 7:
        return [(0, 0, 0, 256, 'G'), (13, 256, 0, 32, 'D'),
                (14, 288, 0, 160, 'A'), (15, 448, 96, 160, 'B')]
    return [(0, 0, 0, 256, 'G'), (2 * r - 1, 256, 0, 32, 'D'),
            (2 * r, 288, 0, 160, 'A'), (2 * r + 1, 448, 96, 160, 'B'),
            (2 * r + 2, 608, 224, 32, 'C')]


CHUNKS = [chunk_plan(r) for r in range(8)]
WR = [sum(c[3] for c in CHUNKS[r]) for r in range(8)]  # 448 / 640 / 608
A_COLS = 448
# mask packing: one column-block per r-class, matching ex layout
MOFF = {0: 640, 7: 1088}  # interior at 0
MASK_W = 640 + 448 + 608


def mask_off(r):
    return MOFF.get(r, 0)


def build_nc():
    nc = bacc.Bacc()
    xT = nc.dram_tensor("xT", [128, 8, S], BF16, kind="ExternalInput")
    wq = nc.dram_tensor("wq", [128, 8, 512], BF16, kind="ExternalInput")
    wk = nc.dram_tensor("wk", [128, 8, 512], BF16, kind="ExternalInput")
    wv = nc.dram_tensor("wv", [128, 8, 512], BF16, kind="ExternalInput")
    wo = nc.dram_tensor("wo", [128, 4, 1024], BF16, kind="ExternalInput")
    bq_c = nc.dram_tensor("bq_c", [128, 4], F32, kind="ExternalInput")
    bk_c = nc.dram_tensor("bk_c", [128, 4], F32, kind="ExternalInput")
    bv_b = nc.dram_tensor("bv_b", [128, 512], F32, kind="ExternalInput")
    masks = nc.dram_tensor("masks", [128, MASK_W], BF16, kind="ExternalInput")
    out = nc.dram_tensor("out", [128, 8, S], BF16, kind="ExternalOutput")

    with tile.TileContext(nc) as tc:
        with (
            tc.tile_pool(name="pers", bufs=1) as pers,
            tc.tile_pool(name="small", bufs=1) as small,
        ):
            x_sb = pers.tile([128, 8, S], BF16)
            q_sb = pers.tile([128, 4, S], BF16)
            k_sb = pers.tile([128, 4, S], BF16)
            v_sb = pers.tile([128, 16, 520], BF16)
            att_sb = pers.tile([128, 4, S], BF16)
            masks_sb = pers.tile([128, MASK_W], BF16)
            wq_sb = pers.tile([128, 8, 512], BF16)
            wk_sb = pers.tile([128, 8, 512], BF16)
            wv_sb = pers.tile([128, 8, 512], BF16)
            wo_sb = pers.tile([128, 4, 1024], BF16)
            bq_sb = small.tile([128, 4], F32)
            bk_sb = small.tile([128, 4], F32)
            bv_sb = small.tile([128, 512], F32)
            ones_sb = small.tile([1, 128], BF16)
            nc.vector.memset(ones_sb, 1.0)

            # DMA issue order tuned so the first q-projection chain can start
            # ~4us in: wq + biases first, then x for st=0, then the rest.
            nc.sync.dma_start(out=wq_sb, in_=wq.ap())
            nc.sync.dma_start(out=bq_sb, in_=bq_c.ap())
            nc.sync.dma_start(out=bk_sb, in_=bk_c.ap())
            for dc in range(8):
                nc.sync.dma_start(out=x_sb[:, dc, 0:512], in_=xT.ap()[:, dc, 0:512])
            nc.sync.dma_start(out=wk_sb, in_=wk.ap())
            nc.sync.dma_start(out=wv_sb, in_=wv.ap())
            nc.sync.dma_start(out=bv_sb, in_=bv_b.ap())
            for dc in range(8):
                nc.sync.dma_start(out=x_sb[:, dc, 512:1024],
                                  in_=xT.ap()[:, dc, 512:1024])
            nc.sync.dma_start(out=masks_sb, in_=masks.ap())
            for st in (2, 3):
                ssl = slice(st * 512, (st + 1) * 512)
                for dc in range(8):
                    nc.sync.dma_start(out=x_sb[:, dc, ssl], in_=xT.ap()[:, dc, ssl])
            nc.sync.dma_start(out=wo_sb, in_=wo.ap())

            # ---------------- Phase 1: projections (dense, x-resident) -----
            with tc.tile_pool(name="pproj", bufs=4, space="PSUM") as pproj:
                for st in range(4):
                    ssl = slice(st * 512, (st + 1) * 512)
                    for cb in range(4):
                        csl = slice(cb * 128, (cb + 1) * 128)
                        pq = pproj.tile([128, 512], F32, tag="proj",
                                        name=f"pq{st}_{cb}")
                        for dc in range(8):
                            nc.tensor.matmul(
                                pq, wq_sb[:, dc, csl], x_sb[:, dc, ssl],
                                start=(dc == 0), stop=(dc == 7))
                        nc.vector.tensor_scalar(
                            out=q_sb[:, cb, ssl], in0=pq,
                            scalar1=bq_sb[:, cb:cb + 1], scalar2=None,
                            op0=ALU.add)
                    for cb in range(4):
                        csl = slice(cb * 128, (cb + 1) * 128)
                        pk = pproj.tile([128, 512], F32, tag="proj",
                                        name=f"pk{st}_{cb}")
                        for dc in range(8):
                            nc.tensor.matmul(
                                pk, wk_sb[:, dc, csl], x_sb[:, dc, ssl],
                                start=(dc == 0), stop=(dc == 7))
                        nc.vector.tensor_scalar(
                            out=k_sb[:, cb, ssl], in0=pk,
                            scalar1=bk_sb[:, cb:cb + 1], scalar2=None,
                            op0=ALU.add)
                    for s4 in range(4):
                        xsl = slice(st * 512 + s4 * 128, st * 512 + s4 * 128 + 128)
                        pv = pproj.tile([128, 512], F32, tag="proj",
                                        name=f"pv{st}_{s4}")
                        for dc in range(8):
                            nc.tensor.matmul(
                                pv, x_sb[:, dc, xsl], wv_sb[:, dc, :],
                                start=(dc == 0), stop=(dc == 7))
                        sc = st * 4 + s4
                        vview = v_sb[:, sc, :].rearrange("p (h w) -> p h w", h=8)
                        nc.vector.tensor_add(
                            out=vview[:, :, 0:64],
                            in0=pv.rearrange("p (h w) -> p h w", h=8),
                            in1=bv_sb.rearrange("p (h w) -> p h w", h=8))
                        nc.vector.memset(vview[:, :, 64:65], 1.0)

            # ---------------- Phase 2: attention pipeline ------------------
            with (
                tc.tile_pool(name="psctA", bufs=2, space="PSUM") as psctA,
                tc.tile_pool(name="psctB", bufs=2, space="PSUM") as psctB,
                tc.tile_pool(name="paug", bufs=2, space="PSUM") as paug,
                # bc (normalizer broadcast) and po (o-proj) share one
                # 2-slot rotation: 8 PSUM banks total, and their lifetimes
                # interleave without stalls.
                tc.tile_pool(name="pmix", bufs=2, space="PSUM") as pmix,
                tc.tile_pool(name="epool", bufs=4) as epool,
                tc.tile_pool(name="rpool", bufs=2) as rpool,
                tc.tile_pool(name="opool", bufs=3) as opool,
            ):
                PAIRS = [(r, hp) for r in range(8) for hp in range(4)]
                state = {}  # j -> dict of tiles

                def emit_scores(j):
                    r, hp = PAIRS[j]
                    st = {}
                    b_cols = WR[r] - A_COLS
                    for hs in (0, 64):
                        st[f"sctA{hs}"] = psctA.tile(
                            [128, 512], F32, tag="sctA", name=f"sA{j}_{hs}")
                        if b_cols > 0:
                            st[f"sctB{hs}"] = psctB.tile(
                                [128, 192], F32, tag="sctB", name=f"sB{j}_{hs}")
                    # interleave h0/h64 chunk by chunk: row-tile concurrency
                    ca = [c for c in CHUNKS[r] if c[1] < A_COLS]
                    cbl = [c for c in CHUNKS[r] if c[1] >= A_COLS]
                    for ci, (sc, exoff, lo, w, mk) in enumerate(ca):
                        for hs in (0, 64):
                            nc.tensor.matmul(
                                st[f"sctA{hs}"][:, exoff:exoff + w],
                                k_sb[hs:hs + 64, hp, sc * 128:sc * 128 + 128],
                                q_sb[hs:hs + 64, hp,
                                     r * 256 + lo:r * 256 + lo + w],
                                start=(ci == 0), stop=(ci == len(ca) - 1),
                                skip_group_check=True)
                    for ci, (sc, exoff, lo, w, mk) in enumerate(cbl):
                        off = exoff - A_COLS
                        for hs in (0, 64):
                            nc.tensor.matmul(
                                st[f"sctB{hs}"][:, off:off + w],
                                k_sb[hs:hs + 64, hp, sc * 128:sc * 128 + 128],
                                q_sb[hs:hs + 64, hp,
                                     r * 256 + lo:r * 256 + lo + w],
                                start=(ci == 0), stop=(ci == len(cbl) - 1),
                                skip_group_check=True)
                    if r == 0:
                        for hs in (0, 64):
                            st[f"gsc{hs}"] = psctB.tile(
                                [128, 64], F32, tag="sctB", name=f"gs{j}_{hs}")
                        for kk in range(16):
                            for hs in (0, 64):
                                nc.tensor.matmul(
                                    st[f"gsc{hs}"][:, 4 * kk:4 * kk + 4],
                                    k_sb[hs:hs + 64, hp,
                                         128 * kk:128 * kk + 128],
                                    q_sb[hs:hs + 64, hp, 0:4],
                                    start=(kk == 0), stop=(kk == 15),
                                    skip_group_check=True)
                    state[j] = st

                def emit_exps(j):
                    r, hp = PAIRS[j]
                    st = state[j]
                    wr = WR[r]
                    b_cols = wr - A_COLS
                    for hs in (0, 64):
                        ex = epool.tile([128, 640], BF16, tag="ex",
                                        name=f"ex{j}_{hs}")
                        st[f"ex{hs}"] = ex
                        nc.scalar.activation(
                            ex[:, 0:A_COLS], st[f"sctA{hs}"][:, 0:A_COLS],
                            AF.Exp, scale=SCALE)
                        if b_cols > 0:
                            nc.scalar.activation(
                                ex[:, A_COLS:wr], st[f"sctB{hs}"][:, 0:b_cols],
                                AF.Exp, scale=SCALE)
                        if r == 0:
                            exg = epool.tile([128, 64], BF16, tag="exg",
                                             name=f"xg{j}_{hs}")
                            st[f"exg{hs}"] = exg
                            nc.scalar.activation(exg, st[f"gsc{hs}"],
                                                 AF.Exp, scale=SCALE)

                def emit_maskmuls(j):
                    r, hp = PAIRS[j]
                    st = state[j]
                    wr = WR[r]
                    mo = mask_off(r)
                    for hs in (0, 64):
                        nc.vector.tensor_mul(
                            out=st[f"ex{hs}"][:, 0:wr],
                            in0=st[f"ex{hs}"][:, 0:wr],
                            in1=masks_sb[:, mo:mo + wr])

                def emit_avs(j):
                    r, hp = PAIRS[j]
                    st = state[j]
                    aug = paug.tile([65, 512], F32, tag="aug", name=f"au{j}")
                    st["aug"] = aug
                    n_ch = len(CHUNKS[r])
                    for hi, hs in enumerate((0, 64)):
                        half = (hs // 64) * 256
                        h65 = (hp * 2 + hs // 64) * 65
                        ex = st[f"ex{hs}"]
                        for ci, (sc, exoff, lo, w, mk) in enumerate(CHUNKS[r]):
                            last = (r != 0 and hi == 1 and ci == n_ch - 1)
                            nc.tensor.matmul(
                                aug[:, half + lo:half + lo + w],
                                v_sb[:, sc, h65:h65 + 65],
                                ex[:, exoff:exoff + w],
                                start=(hi == 0 and ci == 0), stop=last,
                                skip_group_check=True)
                    if r == 0:
                        for hi, hs in enumerate((0, 64)):
                            half = (hs // 64) * 256
                            h65 = (hp * 2 + hs // 64) * 65
                            exg = st[f"exg{hs}"]
                            for kk in range(16):
                                nc.tensor.matmul(
                                    aug[:, half:half + 4],
                                    v_sb[:, kk, h65:h65 + 65],
                                    exg[:, 4 * kk:4 * kk + 4],
                                    start=False,
                                    stop=(hi == 1 and kk == 15),
                                    skip_group_check=True)

                def emit_den(j):
                    st = state[j]
                    den = epool.tile([1, 512], BF16, tag="den", name=f"dn{j}")
                    st["den"] = den
                    nc.vector.tensor_copy(out=den, in_=st["aug"][64:65, :])

                def emit_attcopies(j):
                    r, hp = PAIRS[j]
                    st = state[j]
                    rsl = slice(r * 256, (r + 1) * 256)
                    for hs in (0, 64):
                        half = (hs // 64) * 256
                        nc.scalar.copy(out=att_sb[hs:hs + 64, hp, rsl],
                                       in_=st["aug"][0:64, half:half + 256])

                def emit_bc(j):
                    st = state[j]
                    bc = pmix.tile([128, 256], F32, tag="pobc", name=f"bc{j}")
                    st["bc"] = bc
                    den = st["den"]
                    nc.tensor.matmul(bc[0:64, :], ones_sb[0:1, 0:64],
                                     den[0:1, 0:256], start=True, stop=True)
                    nc.tensor.matmul(bc[64:128, :], ones_sb[0:1, 0:64],
                                     den[0:1, 256:512], start=True, stop=True,
                                     tile_position=(0, 64))
                    del st["den"]

                def emit_recmul(j):
                    r, hp = PAIRS[j]
                    st = state[j]
                    rsl = slice(r * 256, (r + 1) * 256)
                    rec = rpool.tile([128, 256], F32, tag="rec", name=f"rc{j}")
                    nc.vector.reciprocal_approx_fast(out=rec, in_=st["bc"])
                    nc.vector.tensor_mul(
                        out=att_sb[:, hp, rsl], in0=att_sb[:, hp, rsl],
                        in1=rec)
                    state.pop(j, None)

                def oproj_unit(stq, et):
                    # one dense 4-MM N=512 chain: HAM-warming filler spread
                    # through the attention pipeline
                    ssl = slice(stq * 512, (stq + 1) * 512)
                    esl = slice(et * 128, (et + 1) * 128)
                    po = pmix.tile([128, 512], F32, tag="pobc",
                                   name=f"po{stq}_{et}")
                    for cb in range(4):
                        nc.tensor.matmul(
                            po, wo_sb[:, cb, esl], att_sb[:, cb, ssl],
                            start=(cb == 0), stop=(cb == 3))
                    otq = opool.tile([128, 512], BF16, tag="otq",
                                     name=f"otq{stq}_{et}")
                    nc.vector.tensor_copy(out=otq, in_=po)
                    nc.sync.dma_start(out=out.ap()[:, et, ssl], in_=otq)

                for j in range(32):
                    emit_scores(j)
                    if j >= 2:
                        emit_bc(j - 2)
                    if j >= 1:
                        emit_avs(j - 1)
                    if j >= 2:
                        emit_recmul(j - 2)
                    emit_exps(j)
                    emit_maskmuls(j)
                    if j >= 1:
                        emit_den(j - 1)
                        emit_attcopies(j - 1)
                    if j >= 9:
                        u = j - 9
                        oproj_unit(u // 8, u % 8)
                # flush
                emit_avs(31)
                emit_den(31)
                emit_attcopies(31)
                emit_bc(30)
                emit_recmul(30)
                emit_bc(31)
                emit_recmul(31)
                for u in range(23, 32):
                    oproj_unit(u // 8, u % 8)

    nc.compile()
    return nc


def _host_masks():
    p = np.arange(128)[:, None]

    def band(delta, lo, w):
        sl = np.arange(w)[None, :]
        return (np.abs(delta + p - lo - sl) <= 32).astype(np.float32)

    def gcols(w):
        sl = np.arange(w)[None, :]
        return ((p < 4) + 0 * sl).astype(np.float32)

    def special(w):  # r=0 j0: t in [0,128), s=sl
        sl = np.arange(w)[None, :]
        return ((sl >= 4) & ((np.abs(p - sl) <= 32) | (p < 4))).astype(np.float32)

    interior = np.concatenate(
        [gcols(256), band(-128, 0, 32), band(0, 0, 160),
         band(128, 96, 160), band(256, 224, 32)], axis=1)
    r0 = np.concatenate(
        [special(256), band(128, 96, 160), band(256, 224, 32)], axis=1)
    r7 = np.concatenate(
        [gcols(256), band(-128, 0, 32), band(0, 0, 160),
         band(128, 96, 160)], axis=1)
    full = np.concatenate([interior, r0, r7], axis=1)
    assert full.shape == (128, MASK_W)
    return full.astype(ml_dtypes.bfloat16)


_NC = None
_LAST_IN_MAPS = None


def kernel(x, Wq, bq, Wk, bk, Wv, bv, Wo, bo):
    global _NC
    if _NC is None:
        _NC = build_nc()
    nc = _NC
    x = np.asarray(x, np.float32)
    B = x.shape[0]
    bf = ml_dtypes.bfloat16

    def chunked_T(a):  # [R, C] -> [128, C//128, R]; [p, c, r] = a[r, 128c+p]
        at = np.ascontiguousarray(a.T)
        return at.reshape(at.shape[0] // 128, 128, at.shape[1]).transpose(1, 0, 2)

    masks_h = _host_masks()
    in_maps = []
    for core in range(NCORES):
        b, g = core // 2, core % 2
        gs = slice(512 * g, 512 * (g + 1))
        in_maps.append({
            "xT": np.ascontiguousarray(chunked_T(x[b])).astype(bf),
            "wq": np.ascontiguousarray(chunked_T(np.asarray(Wq)[gs, :])).astype(bf),
            "wk": np.ascontiguousarray(chunked_T(np.asarray(Wk)[gs, :])).astype(bf),
            "wv": np.ascontiguousarray(chunked_T(np.asarray(Wv)[gs, :])).astype(bf),
            "wo": np.ascontiguousarray(chunked_T(np.asarray(Wo)[:, gs])).astype(bf),
            "bq_c": np.asarray(bq)[gs].reshape(4, 128).T.copy().astype(np.float32),
            "bk_c": np.asarray(bk)[gs].reshape(4, 128).T.copy().astype(np.float32),
            "bv_b": np.broadcast_to(
                np.asarray(bv)[gs], (128, 512)).copy().astype(np.float32),
            "masks": masks_h,
        })

    global _LAST_IN_MAPS
    _LAST_IN_MAPS = in_maps
    res = run_bass_kernel_spmd(nc, in_maps, list(range(NCORES)))
    out = np.empty((B, S, D), np.float32)
    for b in range(B):
        acc = res.results[2 * b]["out"].astype(np.float32) + \
            res.results[2 * b + 1]["out"].astype(np.float32)
        full_T = acc.transpose(1, 0, 2).reshape(D, S)
        out[b] = full_T.T + np.asarray(bo)[None, :]
    return out
